# revision 20
# baseline (speedup 1.0000x reference)
"""Trainium2 Bass kernel: multi-head attention (B=2, S=2048, H=768, 12 heads x 64).

Sharding: 24 (batch, head) pairs over 8 cores -> 3 heads of one batch per core
(pure data/head parallel, no collectives; outputs gathered host-side).

Design (v9 — PE-overlap + ramp rewrite of v2; 133.6us -> ~125us measured):
  - Host pre-casts to bf16 AND pre-transposes hs (hs^T [H, S]); weights are
    packed PARTITION-MAJOR (p, k, c) per block so each weight DMA is one
    contiguous descriptor per partition (the old (k p c) layout gathered
    256B elements and took ~5us to land, stalling the first chain). hs^T
    chunks are interleaved across both HWDGE queues (SP + ACT).
  - DMA-engine completion lags descriptor-queue time by ~3us, and ~7us of
    runtime prologue precedes user instructions, so the first chain cannot
    start before ~10.7us. Eight N=256 warm-up matmuls on a zeroed scratch
    tile bridge that stretch with continuous PE activity so the HAM
    clock-gate lifts the PE 1.2->2.4 GHz early; zero-matmuls (adding +0
    into the live accumulation) fill the chunk-0 chain's DMA-arrival gaps.
  - Weight columns packed as [Q01 | K01 | K2,Q2 | V]; the two 64-wide
    head-2 projections share one M=128 matmul chain; K2/Q2 are duplicated
    into both partition halves (cheap DVE 4x-mode copies — on GPSIMD these
    took 1864ns each and stalled the first score groups into a HAM
    re-throttle) so head-2 score matmuls can run on either PE row tile.
  - Scores are contract-64 matmuls: heads 0/1 sit in partition halves
    0:64 / 64:128, so bass auto-derives 64x128 row-tiled tile_positions and
    the PE runs h0 ∥ h1 CONCURRENTLY (measured dt_start 4ns). kv tiles are
    processed in PAIRS so the two head-2 matmuls (alternating halves by kv
    parity) also pair up: 6 score matmuls take 3x512-cycle slots, and the
    64-row <-> 128-row array reconfig stalls (~95ns) drop to 4 per pair.
    A 6-score burst would outrun the 5-slot score-PSUM rotation (measured
    +1.6us stalls), so the two s0/s1 pairs straddle the first ctx batch.
  - Each per-head V tile carries a ones column (M=65) so the ctx matmul
    computes the softmax denominator in PSUM row 64 for free. ctx matmuls
    (contract 128) run in full-array mode, batched per kv-tile pair.
  - Exponentials split across ACT (plain EXP) and DVE (Schraudolph
    bit-trick, u16 = round(x*(128*log2e*0.125) + (127*128 - 7.5)) bitcast
    to bf16 == exp(0.125*x), ~1.8% rms — softmax tolerates it). 3/3 per
    group steady-state, 4/2 in window 0 where DVE also owns most
    projection casts (V casts merged into ONE strided DVE cast per kv
    tile; kt01's cast moved to ACT to balance window-0 load).
  - ctx accumulates per kv-tile lagging scores by two kv tiles; softmax
    division is deferred to the host (ctx^T + denominators stream out as
    [65, 512] bf16 tiles). bq optionally added in-kernel; bk cancels in
    softmax; bv added host-side.

Steady state measured: 6 score matmuls (3 concurrent row-tile pairs) + 6
ctx matmuls per kv-tile pair = ~2392ns/pair with zero PE gaps >300ns after
warm-up; the PE streams back-to-back for the whole kernel. Remaining time
is the fixed prologue (~7us), DMA-lag ramp, and the M=65 ctx output-rate
limit (the ones-column denominator blocks 2-way column tiling; every
alternative denominator scheme costs more PE time than it saves).
"""

import sys

sys.path.insert(0, "/opt/trn_rl_repo")

import numpy as np
import ml_dtypes

from concourse import bacc, mybir, tile
from concourse.bass_utils import run_bass_kernel_spmd

F32 = mybir.dt.float32
BF16 = mybir.dt.bfloat16
U16 = mybir.dt.uint16
EXP = mybir.ActivationFunctionType.Exp
CPY = mybir.ActivationFunctionType.Copy
AOp = mybir.AluOpType

B, S, H, NH, HD = 2, 2048, 768, 12, 64
NC = 8  # cores
HPC = 3  # heads per core
DL = HPC * HD  # 192 local columns
KT = H // 128  # 6 contraction tiles
NT = S // 128  # 16 kv tiles
QC = 512  # query chunk
NQC = S // QC  # 4
MJ = 3 * DL  # 576 packed weight columns
NWARM = 8  # HAM warm-up matmuls bridging the DMA ramp

# Schraudolph exp-as-bits constants (DVE offload of exponentials):
# u16 = round(s * SCH_A + SCH_B); u16 bits viewed as bf16 ~= exp(0.125 * s).
SCH_A = (128.0 / float(np.log(2.0))) * 0.125
SCH_B = 127.0 * 128.0 - 7.5

_CACHE = {}


def _build(use_qbias: bool):
    nc = bacc.Bacc("TRN2", target_bir_lowering=False, debug=False)
    hst_d = nc.dram_tensor("hst", [H, S], BF16, kind="ExternalInput").ap()
    wf_d = nc.dram_tensor("wf", [H * MJ], BF16, kind="ExternalInput").ap()
    out_d = nc.dram_tensor("out", [NQC, HPC, HD + 1, QC], BF16,
                           kind="ExternalOutput").ap()
    if use_qbias:
        bq_d = nc.dram_tensor("bq", [DL], F32, kind="ExternalInput").ap()

    ts = tile.bass.ts

    with tile.TileContext(nc) as tc:
        with tc.tile_pool(name="const", bufs=1) as cpool, \
             tc.tile_pool(name="qkv_sb", bufs=1) as qkv_pool, \
             tc.tile_pool(name="et_p", bufs=2) as et_pool, \
             tc.tile_pool(name="cs_p", bufs=2) as cs_pool, \
             tc.tile_pool(name="sc_ps", bufs=5, space="PSUM") as sc_pool, \
             tc.tile_pool(name="cx_ps", bufs=1, space="PSUM") as cx_pool:

            wk01 = qkv_pool.tile([128, KT, 128], BF16)
            wq01 = qkv_pool.tile([128, KT, 128], BF16)
            wkq2 = qkv_pool.tile([128, KT, 128], BF16)
            wv = qkv_pool.tile([128, KT, DL], BF16)
            hsT = qkv_pool.tile([128, KT, S], BF16)
            kt01 = qkv_pool.tile([128, S], BF16)
            kt2 = qkv_pool.tile([128, S], BF16)
            qt01 = qkv_pool.tile([128, S], BF16)
            qt2 = qkv_pool.tile([128, S], BF16)
            v1 = qkv_pool.tile([128, NT, HPC, HD + 1], BF16)
            wscr = cpool.tile([128, QC], BF16)

            # ---- input DMA, interleaved across both HWDGE queues ----
            # wf is packed host-side PARTITION-MAJOR per block (p, k, c) so
            # each weight DMA is one contiguous run per partition (the v2/v3
            # (k p c) layout forced 256B descriptors and the K01 gather took
            # ~5us to land, stalling the first chain to ~12us). The ACT queue
            # carries the weights and chunk-0's k3/k4; SP the rest.
            def w_dma(wt, blk, wd):
                nc.scalar.dma_start(
                    wt[:, :, :],
                    wf_d[blk * H * 128 : blk * H * 128 + H * wd].rearrange(
                        "(p k c) -> p k c", p=128, k=KT, c=wd))

            def hst_dma(eng, c, k):
                eng.dma_start(hsT[:, k, ts(c, QC)], hst_d[ts(k, 128), ts(c, QC)])

            # first half-memset completes ~130ns in so the warm-up matmuls
            # can issue before the first weight DMA lands
            nc.vector.memset(wscr[:, 0:256], 0.0)
            nc.vector.memset(wscr[:, 256:QC], 0.0)
            nc.vector.memset(v1[:, :, :, HD : HD + 1], 1.0)

            w_dma(wk01, 0, 128)
            for k in (0, 1, 2):
                hst_dma(nc.sync, 0, k)
            hst_dma(nc.scalar, 0, 3)
            w_dma(wkq2, 2, 128)
            hst_dma(nc.scalar, 0, 4)
            hst_dma(nc.sync, 0, 5)
            w_dma(wq01, 1, 128)
            w_dma(wv, 3, DL)
            for k in range(KT):
                hst_dma(nc.sync, 1, k)
            for k in range(KT):
                hst_dma(nc.scalar, 2, k)
            for k in range(KT):
                hst_dma(nc.sync, 3, k)
            if use_qbias:
                bq_sb = cpool.tile([128, 2, 1], F32)
                nc.sync.dma_start(
                    bq_sb[0:128, 0, :], bq_d[0:128].rearrange("(p o) -> p o", o=1))
                nc.sync.dma_start(
                    bq_sb[0:64, 1, :], bq_d[128:192].rearrange("(p o) -> p o", o=1))

            # ---- HAM warm-up: dummy matmuls under the DMA ramp ----
            # N=256 so eight of them bridge the ~7.7us (first memset done) to
            # ~10.7us (first hsT tile lands) stretch with CONTINUOUS PE
            # activity: the HAM clock-gate only lifts to 2.4GHz after a full
            # free-running 3.4us window of busy PE.
            for i in range(NWARM):
                wp = sc_pool.tile([128, QC], F32, tag="sc", name=f"warm{i}")
                nc.tensor.matmul(wp[:, 0:256], wscr[:, 0:128], wscr[:, 0:256],
                                 start=True, stop=True)

            # ---- projections (weight cols packed host-side) ----
            def k01(c, zmm=False):
                ps = sc_pool.tile([128, QC], F32, tag="sc", name=f"k01p{c}")
                for k in range(KT):
                    nc.tensor.matmul(ps[:], wk01[:, k, :],
                                     hsT[:, k, ts(c, QC)],
                                     start=(k == 0), stop=(k == KT - 1))
                    if zmm and k < KT - 1:
                        # zero-matmul filler: wscr is all zeros, so this
                        # accumulates +0 into the chain's PSUM. It keeps the
                        # PE busy through the chunk-0 DMA-arrival gaps so the
                        # HAM clock-gate sees continuous activity and lifts
                        # the PE to 2.4GHz ~6us earlier.
                        nc.tensor.matmul(ps[:], wscr[:, 0:128], wscr[:],
                                         start=False, stop=False)
                nc.scalar.activation(kt01[:, ts(c, QC)], ps[:], CPY)

            def k2q2(c):
                ps = sc_pool.tile([128, QC], F32, tag="sc", name=f"k2q2p{c}")
                for k in range(KT):
                    nc.tensor.matmul(ps[:], wkq2[:, k, :],
                                     hsT[:, k, ts(c, QC)],
                                     start=(k == 0), stop=(k == KT - 1))
                # lower halves cast from PSUM on DVE; the upper-half dups are
                # packed bf16 SBUF->SBUF copies that hit DVE 4x mode (~194ns)
                # — on GPSIMD they took 1864ns each and stalled the first
                # score groups (which cascaded into a HAM re-throttle).
                nc.vector.tensor_copy(kt2[0:64, ts(c, QC)], ps[0:64, :])
                nc.vector.tensor_copy(kt2[64:128, ts(c, QC)],
                                      kt2[0:64, ts(c, QC)])
                dq = qt2[0:64, ts(c, QC)]
                if use_qbias:
                    nc.vector.tensor_scalar_add(dq, ps[64:128, :],
                                                bq_sb[0:64, 1, :])
                else:
                    nc.vector.tensor_copy(dq, ps[64:128, :])
                nc.vector.tensor_copy(qt2[64:128, ts(c, QC)], dq)

            def q01(c):
                ps = sc_pool.tile([128, QC], F32, tag="sc", name=f"q01p{c}")
                for k in range(KT):
                    nc.tensor.matmul(ps[:], wq01[:, k, :],
                                     hsT[:, k, ts(c, QC)],
                                     start=(k == 0), stop=(k == KT - 1))
                if use_qbias:
                    nc.vector.tensor_scalar_add(
                        qt01[:, ts(c, QC)], ps[:], bq_sb[0:128, 0, :])
                else:
                    nc.vector.tensor_copy(qt01[:, ts(c, QC)], ps[:])

            def vproj(t):
                ps = sc_pool.tile([128, DL], F32, tag="sc", name=f"vp{t}")
                for k in range(KT):
                    nc.tensor.matmul(ps[:], hsT[:, k, ts(t, 128)],
                                     wv[:, k, :],
                                     start=(k == 0), stop=(k == KT - 1))
                # one strided cast for all three heads (dst skips the ones col)
                nc.vector.tensor_copy(v1[:, t, :, 0:HD], ps[:])

            # ---- scores + exp ----
            ets = {}

            def alloc_et(qc):
                ets[qc] = (
                    et_pool.tile([128, NT, 2, QC], BF16, tag="et01",
                                 name=f"et01_{qc}"),
                    et_pool.tile([128, NT, QC], BF16, tag="et2",
                                 name=f"et2_{qc}"),
                )

            def _exp(eng, et_ap, ps):
                # eng: 'a' = ACT exp; 'v' = Schraudolph bits on DVE
                if eng == 'a':
                    nc.scalar.activation(et_ap, ps[:], EXP, scale=0.125)
                else:
                    nc.vector.tensor_scalar(et_ap.bitcast(U16), ps[:],
                                            SCH_A, SCH_B, AOp.mult, AOp.add)

            def score_s01(qc, t, e0, e1):
                et01, _ = ets[qc]
                for h, eng in ((0, e0), (1, e1)):
                    ps = sc_pool.tile([128, QC], F32, tag="sc",
                                      name=f"s{qc}_{t}_{h}")
                    nc.tensor.matmul(
                        ps[:],
                        kt01[h * 64 : h * 64 + 64, ts(t, 128)],
                        qt01[h * 64 : h * 64 + 64, ts(qc, QC)],
                        start=True, stop=True)
                    _exp(eng, et01[:, t, h, :], ps)

            def score_s2(qc, t, e2):
                _, et2 = ets[qc]
                hh = t % 2
                ps2 = sc_pool.tile([128, QC], F32, tag="sc",
                                   name=f"s{qc}_{t}_2")
                nc.tensor.matmul(
                    ps2[:],
                    kt2[hh * 64 : hh * 64 + 64, ts(t, 128)],
                    qt2[hh * 64 : hh * 64 + 64, ts(qc, QC)],
                    start=True, stop=True)
                _exp(e2, et2[:, t, :], ps2)

            # ---- ctx (+denominator via the V ones-column) ----
            cxs = {}

            def ctx_alloc(qc):
                cxs[qc] = [
                    cx_pool.tile([HD + 1, QC], F32, tag=f"cx{h}",
                                 name=f"cx{qc}_{h}")
                    for h in range(HPC)
                ]

            def ctx_partial(qc, t, start, stop):
                et01, et2 = ets[qc]
                for h in range(HPC):
                    rhs = et2[:, t, :] if h == 2 else et01[:, t, h, :]
                    nc.tensor.matmul(cxs[qc][h][:], v1[:, t, h, :], rhs,
                                     start=start, stop=stop)

            def drain(qc):
                # split across engines: ACT (idle at drain points) casts and
                # ships head 0 on its own HWDGE queue; DVE/sync take the rest.
                # In the tail (last window) ACT also takes head 2 so the two
                # remaining casts run concurrently instead of serially on DVE.
                last = qc == NQC - 1
                for h in range(HPC):
                    cs = cs_pool.tile([HD + 1, QC], BF16, tag=f"cs{h}",
                                      name=f"cs{qc}_{h}")
                    if h == 0 or (last and h == 2):
                        nc.scalar.activation(cs[:], cxs[qc][h][:], CPY)
                        nc.scalar.dma_start(out_d[qc, h, :, :], cs[:])
                    else:
                        nc.vector.tensor_copy(cs[:], cxs[qc][h][:])
                        nc.sync.dma_start(out_d[qc, h, :, :], cs[:])

            # ---- schedule ----
            # kv tiles processed in PAIRS (t, t+1). Per group: the four
            # 64-row-mode score matmuls s2(t) || s2(t+1) || s0(t) || s1(t)
            # pipeline on alternating PE row tiles, then a 128-row-mode ctx
            # batch for tile t-2, then s0/s1 of t+1, then ctx for t-1. The
            # 5-slot score-PSUM rotation absorbs the 6 in-flight tiles: the
            # 6th allocation (s1(t+1)) issues ~1.2us into the group, after
            # the group's first exp has freed its slot.
            # Window 0 interleaves the projections per 512-col chunk, all in
            # full-array mode ahead of that chunk's score groups.
            def group(qc, t, engs, pqc):
                # engs: 6 chars: s2(t), s2(t+1), s0(t), s1(t), s0(t+1), s1(t+1)
                # pqc: the window owning ctx tiles t-2/t-1 (qc-1 wrap at t=0)
                # Four score matmuls (two concurrent row-tile pairs), a ctx
                # batch, the second s0/s1 pair, the second ctx batch. A
                # 6-score burst would outrun the 5-slot score-PSUM rotation
                # (measured: 6th allocation stalls ~1.6us on its slot's exp).
                score_s2(qc, t, engs[0])
                score_s2(qc, t + 1, engs[1])
                score_s01(qc, t, engs[2], engs[3])
                j = (t - 2) % NT
                if pqc is not None:
                    ctx_partial(pqc, j, start=(j == 0), stop=False)
                score_s01(qc, t + 1, engs[4], engs[5])
                if pqc is not None:
                    ctx_partial(pqc, j + 1, start=(j + 1 == 0),
                                stop=(j + 1 == NT - 1))

            alloc_et(0)
            ctx_alloc(0)
            for c in range(NQC):
                k01(c, zmm=(c == 0))
                k2q2(c)
                if c < 2:
                    q01(c)
                for t in range(4 * c, 4 * c + 4):
                    vproj(t)
                for t in (4 * c, 4 * c + 2):
                    group(0, t, 'aavaav', 0 if t >= 2 else None)

            for qc in range(1, NQC):
                alloc_et(qc)
                if qc < 3:
                    q01(qc + 1)
                for t in range(0, NT, 2):
                    # drain before tile 2's scores so its casts clear the
                    # ACT/DVE FIFOs ahead of the fresh exps — ctx(qc, 0)
                    # WARs on them and would otherwise stall the PE.
                    if t == 2:
                        drain(qc - 1)
                        ctx_alloc(qc)
                    group(qc, t, 'avavav', qc - 1 if t == 0 else qc)
            for t in range(NT - 2, NT):
                ctx_partial(NQC - 1, t, start=False, stop=(t == NT - 1))
            drain(NQC - 1)

    nc.compile()
    return nc


def _get(use_qbias: bool):
    key = use_qbias
    if key not in _CACHE:
        _CACHE[key] = _build(use_qbias)
    return _CACHE[key]


def _make_in_maps(hidden_states, Wq, bq, Wk, Wv, use_qbias):
    in_maps = []
    for i in range(NC):
        b, g = divmod(i, NC // B)
        c0 = g * DL
        def pmaj(blk):
            # [H, cols] -> partition-major (p, k, c) flat, one contiguous
            # DMA descriptor per partition
            cols = blk.shape[1]
            return (np.ascontiguousarray(blk).astype(ml_dtypes.bfloat16)
                    .reshape(KT, 128, cols).transpose(1, 0, 2).ravel())

        wf = np.concatenate(
            [
                pmaj(blk)
                for blk in (
                    Wk[:, c0 : c0 + 128],                   # K01 (first DMA)
                    Wq[:, c0 : c0 + 128],                   # Q01
                    np.concatenate(                          # K2 | Q2
                        [Wk[:, c0 + 128 : c0 + 192],
                         Wq[:, c0 + 128 : c0 + 192]], axis=1),
                    Wv[:, c0 : c0 + DL],                    # V
                )
            ]
        )
        m = {
            "hst": np.ascontiguousarray(hidden_states[b].T).astype(
                ml_dtypes.bfloat16),
            "wf": wf,
        }
        if use_qbias:
            m["bq"] = np.ascontiguousarray(bq[c0 : c0 + DL], dtype=np.float32)
        in_maps.append(m)
    return in_maps


def _run(inputs, trace=False):
    hidden_states = np.asarray(inputs["hidden_states"], dtype=np.float32)
    Wq = np.asarray(inputs["Wq"], dtype=np.float32)
    Wk = np.asarray(inputs["Wk"], dtype=np.float32)
    Wv = np.asarray(inputs["Wv"], dtype=np.float32)
    bq = np.asarray(inputs["bq"], dtype=np.float32)
    bv = np.asarray(inputs["bv"], dtype=np.float32)
    # bk is intentionally unused: softmax over the kv axis cancels any
    # per-query constant, and q_i . bk is constant along kv.
    assert hidden_states.shape == (B, S, H)
    use_qbias = bool(np.any(bq))
    nc = _get(use_qbias)
    in_maps = _make_in_maps(hidden_states, Wq, bq, Wk, Wv, use_qbias)
    res = run_bass_kernel_spmd(nc, in_maps, core_ids=list(range(NC)), trace=trace)
    out = np.empty((B, S, H), dtype=np.float32)
    for i in range(NC):
        b, g = divmod(i, NC // B)
        c0 = g * DL
        arr = np.asarray(res.results[i]["out"]).astype(np.float32)
        ctx = arr[:, :, 0:HD, :]           # [NQC, HPC, HD, QC]
        den = arr[:, :, HD, :]             # [NQC, HPC, QC]
        blk = ctx / den[:, :, None, :]
        out[b, :, c0 : c0 + DL] = (
            blk.transpose(0, 3, 1, 2).reshape(S, DL) + bv[c0 : c0 + DL])
    return out, res


def kernel(**inputs) -> np.ndarray:
    out, _ = _run(inputs, trace=False)
    return out
